# revision 19
# baseline (speedup 1.0000x reference)
"""MAB (multihead attention block) TRN2 kernel, v4.

Sharding: 8 cores = batch (4) x query-half (2). Each core computes its
[1024, 256] output slice with zero cross-core communication (K/V
projections are recomputed by the 2 cores sharing a batch).

v4 design (driven by the TimelineSim cost model):
- The exp stream is SPLIT between Act (AF.Exp -> fp8) and DVE (one-op
  Schraudolph: f32 psum -> uint8 = fp8e4m3 bit pattern of ~exp(x)).
  Both produce fp8 tiles in the shared uts buffer, so A@V replays them
  uniformly as fp8 DoubleRow matmuls (2 key-chunks per matmul at 0.5
  cycles/row).
- A@V epilogue is a single fused scalar_tensor_tensor per head:
  O0n[head slice] = acc*recip + q_nat, with accum_out emitting the
  row-sum partial for LN0 for free.
- PSUM: a 3-deep rotation of [128,1024] score slots (6 banks) shared by
  the PE filler and both exp consumers, plus a 2-deep [128,512] thunk
  pool (2 banks). The tail reuses the idle score slots so all 8 A@V
  groups of the last block accumulate concurrently.
- FFN computes its output in NATURAL layout (stationary = O1T tokens,
  moving = WoT) so the second transpose set disappears.
- Pool (gpsimd) absorbs the SBUF-side elementwise work: LN squares and
  grouped reduces, LN applies, FFN residual adds.
"""

import numpy as np

import concourse.mybir as mybir
import concourse.tile as tile
from concourse import bacc
from concourse.bass_utils import run_bass_kernel_spmd

F32 = mybir.dt.float32
F32R = mybir.dt.float32r
BF16 = mybir.dt.bfloat16
FP8 = mybir.dt.float8e4
U8 = mybir.dt.uint8
I32 = mybir.dt.int32
AF = mybir.ActivationFunctionType
ALU = mybir.AluOpType
PM = mybir.MatmulPerfMode

B, NQ, NK, D = 4, 2048, 2048, 256
H, DH = 4, 64
S = NQ // 2          # queries per core
ET = D // 128        # feature tiles
KT = NK // 128       # key chunks of 128
KB = NK // 512       # key blocks of 512
QB = S // 512        # query blocks of 512
TT = S // 128        # token tiles of 128
NT = 2 * KT          # score tiles per block (hp x kt)
EPS = 1e-5
SCALE = 1.0 / np.sqrt(D)

# Schraudolph fp8 exp constants for the DVE tiles:
# uint8 bits = rint(x*SCALE * 8/ln2 + 7*8 - c); bitcast fp8e4m3 ~ exp(x*SCALE)
A_U8 = float(SCALE * 8.0 / np.log(2.0))
B_U8 = 56.0 - 0.34

# per-block score tiles run on DVE (rest on Act). Tile index t = hp*KT+kt.
# Kept away from block edges so slot-rotation WARs near the block boundary
# only depend on the (faster-draining) Act queue.
DVE_TILES = (3, 6, 10, 14, 18, 21, 24, 27)

_CACHE = {}


def _build(flags):
    (use_bq, use_bk, use_bv, use_bo, use_g0, use_g1) = flags
    nc = bacc.Bacc(None, target_bir_lowering=False)

    dQ = nc.dram_tensor("QT", [D, S], F32, kind="ExternalInput")
    dK = nc.dram_tensor("KT", [D, NK], F32, kind="ExternalInput")
    dW = {w: nc.dram_tensor(w, [D, D], F32, kind="ExternalInput")
          for w in ("WqT", "WkT", "WvT", "WoT")}
    dV = {v: nc.dram_tensor(v, [D], F32, kind="ExternalInput")
          for v in ("bq", "bk", "bv", "bo", "g0", "b0", "g1", "b1")}
    dI = nc.dram_tensor("IDN", [128, 128], F32, kind="ExternalInput")
    dO = nc.dram_tensor("Out", [S, D], F32, kind="ExternalOutput")

    any_flag = any(flags)

    with tile.TileContext(nc) as tc:
        with (
            tc.tile_pool(name="const", bufs=1) as cpool,
            tc.tile_pool(name="acts", bufs=1) as apool,
            tc.tile_pool(name="small", bufs=6) as smp,
            tc.tile_pool(name="stage", bufs=1) as stpool,
        ):
            # ---------------- DMAs (issue order = need order) ----------------
            # Wq/Q0 on the SP HWDGE queue; K0/Wk via the Pool SWDGE path so
            # the two first-needed pairs transfer concurrently (HWDGE is a
            # single serializing device).
            warm = cpool.tile([128, 128], BF16)
            nc.gpsimd.memset(warm[:], 0.25)
            w0 = {}
            for w in ("WqT", "WkT"):
                w0[w] = cpool.tile([128, ET, D], F32, name=f"{w}0")
            QT0 = stpool.tile([128, ET, S], F32)
            KT0 = stpool.tile([128, ET, NK], F32)
            qv = dQ.rearrange("(dt d) s -> d dt s", d=128)
            kv = dK.rearrange("(dt d) k -> d dt k", d=128)
            nc.sync.dma_start(w0["WqT"][:], dW["WqT"].rearrange("(dt d) e -> d dt e", d=128))
            nc.gpsimd.dma_start(KT0[:, :, 0:512], kv[:, :, 0:512])
            nc.sync.dma_start(QT0[:, :, 0:512], qv[:, :, 0:512])
            nc.gpsimd.dma_start(w0["WkT"][:], dW["WkT"].rearrange("(dt d) e -> d dt e", d=128))
            nc.sync.dma_start(QT0[:, :, 512:1024], qv[:, :, 512:1024])
            for kb in range(1, KB):
                ksl = slice(kb * 512, (kb + 1) * 512)
                nc.sync.dma_start(KT0[:, :, ksl], kv[:, :, ksl])
            w0["WvT"] = cpool.tile([128, ET, D], F32, name="WvT0")
            nc.sync.dma_start(w0["WvT"][:], dW["WvT"].rearrange("(dt d) e -> d dt e", d=128))
            w0["WoT"] = cpool.tile([128, ET, D], F32, name="WoT0")
            nc.sync.dma_start(w0["WoT"][:], dW["WoT"].rearrange("(dt d) e -> d dt e", d=128))
            idn = cpool.tile([128, 128], F32)
            nc.sync.dma_start(idn[:], dI[:, :])

            vrows = {}
            vcols = {}
            if any_flag:
                onesr1 = cpool.tile([1, 128], F32R)
                o1f = cpool.tile([1, 128], F32)
                nc.vector.memset(o1f[:], 1.0)
                nc.vector.tensor_copy(onesr1[:], o1f[:])
            if use_bq or use_bv:
                # softmax rows sum to 1 => attention(v + bv) == attention(v)
                # + bv; both bq and bv fold into the natural-layout q via a
                # rank-1 matmul term.
                bq_row = cpool.tile([1, D], F32)
                bv_row = cpool.tile([1, D], F32)
                nc.sync.dma_start(bq_row[:], dV["bq"][None, :])
                nc.sync.dma_start(bv_row[:], dV["bv"][None, :])
                bqv = cpool.tile([1, D], F32R)
                s0 = cpool.tile([1, D], F32)
                nc.vector.tensor_tensor(out=s0[:], in0=bq_row[:], in1=bv_row[:], op=ALU.add)
                nc.vector.tensor_copy(bqv[:], s0[:])
                vrows["bqv"] = bqv
            for nm, use in (("bq", use_bq), ("bk", use_bk)):
                if use:
                    t = cpool.tile([128, ET], F32, name=f"{nm}c")
                    nc.sync.dma_start(t[:], dV[nm].rearrange("(et e) -> e et", e=128))
                    vcols[nm] = t
            for nm, use in (("bo", use_bo), ("g0", use_g0), ("b0", use_g0),
                            ("g1", use_g1), ("b1", use_g1)):
                if use:
                    t0 = cpool.tile([1, D], F32, name=f"{nm}row0")
                    nc.sync.dma_start(t0[:], dV[nm][None, :])
                    t = cpool.tile([1, D], F32R, name=f"{nm}row")
                    nc.vector.tensor_copy(t[:], t0[:])
                    vrows[nm] = t

            # ---------------- rounding copies (f32 -> f32r) ----------------
            # DVE gets 2x_2p on SBUF->SBUF copies; first-needed chunks on
            # DVE, the rest on Pool so nothing head-of-line-blocks the ramp.
            w_r = {}
            for w in ("WqT", "WkT"):
                w_r[w] = cpool.tile([128, ET, D], F32R, name=w)
                nc.vector.tensor_copy(w_r[w][:], w0[w][:])
            QTt = stpool.tile([128, ET, S], F32R)
            KTt = stpool.tile([128, ET, NK], F32R)
            nc.vector.tensor_copy(QTt[:, :, 0:512], QT0[:, :, 0:512])
            nc.vector.tensor_copy(KTt[:, :, 0:512], KT0[:, :, 0:512])
            nc.gpsimd.tensor_copy(KTt[:, :, 512:1024], KT0[:, :, 512:1024])
            nc.gpsimd.tensor_copy(QTt[:, :, 512:1024], QT0[:, :, 512:1024])
            nc.gpsimd.tensor_copy(KTt[:, :, 1024:1536], KT0[:, :, 1024:1536])
            nc.gpsimd.tensor_copy(KTt[:, :, 1536:2048], KT0[:, :, 1536:2048])
            w_r["WvT"] = cpool.tile([128, ET, D], F32R, name="WvT")
            nc.gpsimd.tensor_copy(w_r["WvT"][:], w0["WvT"][:])
            w_r["WoT"] = cpool.tile([128, ET, D], F32R, name="WoT")
            nc.gpsimd.tensor_copy(w_r["WoT"][:], w0["WoT"][:])

            # ---------------- activation tiles ----------------
            kT_bf = apool.tile([128, ET, NK], BF16)
            qT_bf = apool.tile([128, ET, S], BF16)
            v_sb = apool.tile([128, KT, H, DH + 1], FP8)
            q_nat = apool.tile([128, TT, D], F32)
            O0n = apool.tile([128, TT, D], F32)
            O1n = apool.tile([128, TT, D], F32)
            O2n = apool.tile([128, TT, D], F32)
            O3n = apool.tile([128, TT, D], F32)
            O1T = apool.tile([128, ET, S], F32R)
            rfn = apool.tile([128, TT, D], F32)     # relu(FFN) natural
            scr0 = apool.tile([128, 4, D], F32)     # LN0 squares scratch
            scr1 = apool.tile([128, 4, D], F32)     # LN1 squares scratch
            uts = apool.tile([128, NT, 1024], FP8)  # exp tiles, one block

            nc.gpsimd.memset(v_sb[:, :, :, DH:DH + 1], 1.0)

            # ---------------- LN helpers (natural layout) ----------
            def rsqrt_chain(eng, var, tag, n):
                """Quake rsqrt + 1 Newton iteration; returns f32 AP."""
                yq = smp.tile([128, n], I32, name=f"yq{tag}", tag=f"yq{n}{tag[-1]}")
                eng.tensor_scalar(yq[:], var.bitcast(I32), 1, None,
                                  ALU.logical_shift_right)
                eng.tensor_scalar(yq[:], yq[:], -1, None, ALU.bitwise_xor)
                eng.tensor_scalar(yq[:], yq[:], 0x5f3759e0, None, ALU.add)
                half = smp.tile([128, n], F32, name=f"hf{tag}", tag=f"hf{n}{tag[-1]}")
                eng.tensor_scalar_mul(half[:], var, 0.5)
                yf = yq[:].bitcast(F32)
                t2 = smp.tile([128, n], F32, name=f"t2{tag}", tag=f"t2{n}{tag[-1]}")
                eng.tensor_tensor(out=t2[:], in0=yf, in1=yf, op=ALU.mult)
                eng.tensor_tensor(out=t2[:], in0=t2[:], in1=half[:], op=ALU.mult)
                eng.tensor_scalar(t2[:], t2[:], -1.0, 1.5, ALU.mult, ALU.add)
                eng.tensor_tensor(out=yq[:].bitcast(F32), in0=yf, in1=t2[:], op=ALU.mult)
                return yf

            def ln_finish(x, y, qb, gname, bname, use_g, tag, sum4, sumsq4,
                          store=False):
                """Batch LN finish for the 4 sub-tiles of block qb (DVE chain
                + Pool applies)."""
                negmu = smp.tile([128, 4], F32, name=f"nmu{tag}", tag="nmu4")
                nc.vector.tensor_scalar_mul(negmu[:], sum4[:], -1.0 / D)
                var4 = smp.tile([128, 4], F32, name=f"var{tag}", tag="var4")
                nc.vector.tensor_tensor(out=var4[:], in0=negmu[:], in1=negmu[:], op=ALU.mult)
                nc.vector.tensor_scalar_mul(sumsq4[:], sumsq4[:], 1.0 / D)
                nc.vector.tensor_tensor(out=var4[:], in0=sumsq4[:], in1=var4[:], op=ALU.subtract)
                nc.vector.tensor_scalar_add(var4[:], var4[:], EPS)
                yf = rsqrt_chain(nc.vector, var4[:], tag + "v", 4)
                for sub in range(4):
                    ti = qb * 4 + sub
                    nc.gpsimd.tensor_scalar(y[:, ti, :], x[:, ti, :],
                                            negmu[:, sub:sub + 1], yf[:, sub:sub + 1],
                                            ALU.add, ALU.mult)
                    if store and not use_g:
                        nc.sync.dma_start(
                            dO.rearrange("(t p) e -> p t e", p=128)[:, ti:ti + 1, :],
                            y[:, ti:ti + 1, :])
                if use_g:
                    for sub in range(4):
                        ti = qb * 4 + sub
                        nc.vector.tensor_tensor(out=y[:, ti, :], in0=y[:, ti, :],
                                                in1=brows[gname][:], op=ALU.mult)
                        nc.vector.tensor_tensor(out=y[:, ti, :], in0=y[:, ti, :],
                                                in1=brows[bname][:], op=ALU.add)
                        if store:
                            nc.sync.dma_start(
                                dO.rearrange("(t p) e -> p t e", p=128)[:, ti:ti + 1, :],
                                y[:, ti:ti + 1, :])

            def ln_fin_sub(x, y, qb, sub, tag, sum4, sumsq4, fin_eng, apply_eng,
                           store):
                """Per-sub LN finish for the tail (latency-optimized)."""
                ti = qb * 4 + sub
                sl = slice(sub, sub + 1)
                negmu = smp.tile([128, 1], F32, name=f"nm{tag}{sub}", tag=f"nm{tag[-1]}{sub}")
                fin_eng.tensor_scalar_mul(negmu[:], sum4[:, sl], -1.0 / D)
                var1 = smp.tile([128, 1], F32, name=f"va{tag}{sub}", tag=f"va{tag[-1]}{sub}")
                fin_eng.tensor_tensor(out=var1[:], in0=negmu[:], in1=negmu[:], op=ALU.mult)
                sq = smp.tile([128, 1], F32, name=f"sq{tag}{sub}", tag=f"sq{tag[-1]}{sub}")
                fin_eng.tensor_scalar_mul(sq[:], sumsq4[:, sl], 1.0 / D)
                fin_eng.tensor_tensor(out=var1[:], in0=sq[:], in1=var1[:], op=ALU.subtract)
                fin_eng.tensor_scalar_add(var1[:], var1[:], EPS)
                yf = rsqrt_chain(fin_eng, var1[:], f"{tag}{sub}", 1)
                if apply_eng == "act":
                    nmr = smp.tile([128, 1], F32, name=f"nr{tag}{sub}", tag=f"nr{tag[-1]}{sub}")
                    fin_eng.tensor_tensor(out=nmr[:], in0=negmu[:], in1=yf, op=ALU.mult)
                    nc.scalar.activation(y[:, ti, :], x[:, ti, :], AF.Identity,
                                         bias=nmr[:], scale=yf)
                else:
                    e = nc.gpsimd if apply_eng == "pool" else nc.vector
                    e.tensor_scalar(y[:, ti, :], x[:, ti, :],
                                    negmu[:], yf, ALU.add, ALU.mult)
                if store:
                    # odd subs store via the Pool SWDGE queue so the four
                    # tail stores don't serialize on the single HWDGE device
                    dq = nc.gpsimd if sub % 2 else nc.sync
                    dq.dma_start(
                        dO.rearrange("(t p) e -> p t e", p=128)[:, ti:ti + 1, :],
                        y[:, ti:ti + 1, :])

            # ---------------- projections + attention + MLP ----------------
            with (
                tc.tile_pool(name="scps", bufs=3, space="PSUM") as scps,
                tc.tile_pool(name="paps", bufs=2, space="PSUM") as paps,
            ):
                def pa_tile(name):
                    return paps.tile([128, 512], F32, name=name, tag="pa", bufs=2)

                # --- projections ---
                def qproj(qb, et, act_epi=False):
                    qsl = slice(qb * 512, (qb + 1) * 512)
                    ps = pa_tile(f"qp{et}{qb}")
                    for dt in range(ET):
                        nc.tensor.matmul(
                            ps[:], w_r["WqT"][:, dt, et * 128:(et + 1) * 128],
                            QTt[:, dt, qsl], start=(dt == 0), stop=(dt == ET - 1))
                    dst = qT_bf[:, et, qsl]
                    if use_bq:
                        nc.scalar.activation(dst, ps[:], AF.Identity,
                                             bias=vcols["bq"][:, et:et + 1])
                    elif act_epi:
                        nc.scalar.copy(dst, ps[:])
                    else:
                        nc.vector.tensor_copy(dst, ps[:])

                def kproj(et, kb, act_epi=False):
                    ksl = slice(kb * 512, (kb + 1) * 512)
                    ps = pa_tile(f"kp{kb}{et}")
                    for dt in range(ET):
                        nc.tensor.matmul(
                            ps[:], w_r["WkT"][:, dt, et * 128:(et + 1) * 128],
                            KTt[:, dt, ksl], start=(dt == 0), stop=(dt == ET - 1))
                    dst = kT_bf[:, et, ksl]
                    if use_bk:
                        nc.vector.tensor_scalar_add(dst, ps[:], vcols["bk"][:, et:et + 1])
                    elif act_epi:
                        nc.scalar.copy(dst, ps[:])
                    else:
                        nc.vector.tensor_copy(dst, ps[:])

                def vproj(kt):
                    ps = pa_tile(f"vp{kt}")
                    for dt in range(ET):
                        nc.tensor.matmul(
                            ps[:, 0:D], KTt[:, dt, kt * 128:(kt + 1) * 128],
                            w_r["WvT"][:, dt, :],
                            start=(dt == 0), stop=(dt == ET - 1))
                    nc.vector.tensor_copy(
                        v_sb[:, kt, :, 0:DH],
                        ps[:, 0:D].rearrange("p (h e) -> p h e", e=DH))

                def qnat(t):
                    ps = pa_tile(f"qn{t}")
                    n_steps = ET + (1 if (use_bq or use_bv) else 0)
                    for dt in range(ET):
                        nc.tensor.matmul(
                            ps[:, 0:D], QTt[:, dt, t * 128:(t + 1) * 128],
                            w_r["WqT"][:, dt, :],
                            start=(dt == 0), stop=(dt == n_steps - 1))
                    if use_bq or use_bv:
                        nc.tensor.matmul(ps[:, 0:D], onesr1[:], vrows["bqv"][:],
                                         start=False, stop=True)
                    nc.vector.tensor_copy(q_nat[:, t, :], ps[:, 0:D])

                # --- A@V group: fp8 DoubleRow over key-chunk pairs, with a
                # fused (acc*recip + q_nat) epilogue that writes the O0n head
                # slice and emits the row-sum partial for LN0. ---
                l0p = [None, None]

                def av_group(qb, sub, h, acc=None):
                    hp, hh = divmod(h, 2)
                    ti = qb * 4 + sub
                    if acc is None:
                        acc = pa_tile(f"ac{qb}{sub}{h}")
                    acc = acc[:, 0:DH + 1]
                    cs = hh * 512 + sub * 128
                    for kp in range(KT // 2):
                        nc.tensor.matmul(
                            acc,
                            uts[:, hp * KT + 2 * kp:hp * KT + 2 * kp + 2, cs:cs + 128],
                            v_sb[:, 2 * kp:2 * kp + 2, h, :],
                            start=(kp == 0), stop=(kp == KT // 2 - 1),
                            perf_mode=PM.DoubleRow)
                    rec = smp.tile([128, 1], F32, name=f"rc{qb}{sub}{h}", tag="rec")
                    nc.vector.reciprocal_approx_fast(out=rec[:], in_=acc[:, DH:DH + 1])
                    nc.vector.scalar_tensor_tensor(
                        out=O0n[:, ti, h * DH:(h + 1) * DH],
                        in0=acc[:, 0:DH], scalar=rec[:],
                        in1=q_nat[:, ti, h * DH:(h + 1) * DH],
                        op0=ALU.mult, op1=ALU.add,
                        accum_out=l0p[qb % 2][:, sub * 4 + h:sub * 4 + h + 1])

                def t1(ti, act_epi=False):
                    ps = pa_tile(f"tpA{ti}")
                    for et in range(ET):
                        nc.tensor.transpose(ps[:, et * 128:(et + 1) * 128],
                                            O1n[:, ti, et * 128:(et + 1) * 128], idn[:])
                    src = ps[:, 0:D].rearrange("p (et e) -> p et e", e=128)
                    dst = O1T[:, :, ti * 128:(ti + 1) * 128]
                    if act_epi:
                        nc.scalar.copy(dst, src)
                    else:
                        nc.vector.tensor_copy(dst, src)

                def ffn(ti, relu_eng="dve", ps=None):
                    # natural-layout FFN: stationary = O1T token tile,
                    # moving = WoT -> out [128 tok, 256 feat]
                    if ps is None:
                        ps = pa_tile(f"ff{ti}")
                    for dt in range(ET):
                        nc.tensor.matmul(
                            ps[:, 0:D], O1T[:, dt, ti * 128:(ti + 1) * 128],
                            w_r["WoT"][:, dt, :],
                            start=(dt == 0), stop=(dt == ET - 1))
                    dst = rfn[:, ti, :]
                    if use_bo:
                        nc.vector.tensor_tensor(out=dst, in0=ps[:, 0:D],
                                                in1=brows["bo"][:], op=ALU.add)
                        nc.vector.tensor_scalar_max(dst, dst, 0.0)
                    elif relu_eng == "act":
                        nc.scalar.activation(dst, ps[:, 0:D], AF.Relu, bias=0.0)
                    else:
                        nc.vector.tensor_scalar(dst, ps[:, 0:D], 0.0, 0.0,
                                                ALU.add, ALU.max)

                def residual1(qb, sub, l1s):
                    # O2n = O1n + rfn, with the LN1 row-sum emitted for free
                    ti = qb * 4 + sub
                    nc.gpsimd.scalar_tensor_tensor(
                        out=O2n[:, ti, :], in0=O1n[:, ti, :], scalar=0.0,
                        in1=rfn[:, ti, :], op0=ALU.add, op1=ALU.add,
                        accum_out=l1s[:, sub:sub + 1])

                # broadcast [128, D] rows for bo / LN affine params
                brows = {}
                for nm, use in (("bo", use_bo), ("g0", use_g0), ("b0", use_g0),
                                ("g1", use_g1), ("b1", use_g1)):
                    if use:
                        bt = cpool.tile([128, D], F32, name=f"brow_{nm}")
                        ps = pa_tile(f"bps_{nm}")
                        nc.tensor.matmul(ps[:, 0:D], onesr1[:], vrows[nm][:],
                                         start=True, stop=True)
                        nc.vector.tensor_copy(bt[:], ps[:, 0:D])
                        brows[nm] = bt

                # --- grouped LN stats (mid-block; latency not critical) ---
                def ln_stats0(qb, l0s, l0q):
                    # combine the 16 row-sum partials -> l0s[128,4]
                    nc.vector.tensor_reduce(
                        out=l0s[:], in_=l0p[qb % 2][:].rearrange("p (s h) -> p s h", h=4),
                        axis=mybir.AxisListType.X, op=ALU.add)
                    for sub in range(4):
                        ti = qb * 4 + sub
                        nc.gpsimd.scalar_tensor_tensor(
                            out=scr0[:, sub, :], in0=O0n[:, ti, :], scalar=1.0,
                            in1=O0n[:, ti, :], op0=ALU.mult, op1=ALU.mult,
                            accum_out=l0q[:, sub:sub + 1])

                def ln_stats1(qb, l1q):
                    for sub in range(4):
                        ti = qb * 4 + sub
                        nc.gpsimd.scalar_tensor_tensor(
                            out=scr1[:, sub, :], in0=O2n[:, ti, :], scalar=1.0,
                            in1=O2n[:, ti, :], op0=ALU.mult, op1=ALU.mult,
                            accum_out=l1q[:, sub:sub + 1])

                def block_tail(qb, l0s, l0q, l1s, l1q):
                    """Thunks for block qb emitted at its stream end; drains
                    under the next block's stream."""
                    th = []
                    for sub in range(4):
                        for h in (2, 3):
                            th.append((0.5, lambda qb=qb, sub=sub, h=h: av_group(qb, sub, h)))
                    th.append((0.5, lambda: ln_stats0(qb, l0s, l0q)))
                    th.append((0.4, lambda: ln_finish(
                        O0n, O1n, qb, "g0", "b0", use_g0, f"l0{qb}", l0s, l0q)))
                    for i in range(4):
                        ti = qb * 4 + i
                        th.append((0.5, lambda ti=ti: t1(ti)))
                    for i in range(4):
                        ti = qb * 4 + i
                        th.append((0.5, lambda ti=ti: ffn(ti)))
                        th.append((0.3, lambda qb=qb, i=i: residual1(qb, i, l1s)))
                    th.append((0.5, lambda: ln_stats1(qb, l1q)))

                    def fin():
                        ln_finish(O2n, O3n, qb, "g1", "b1", use_g1,
                                  f"l1{qb}", l1s, l1q)
                        nc.sync.dma_start(
                            dO.rearrange("(t p) e -> p t e", p=128)[:, qb * 4:(qb + 1) * 4, :],
                            O3n[:, qb * 4:(qb + 1) * 4, :])
                    th.append((0.5, fin))
                    return th

                def tail_chain(qb, l0s, l0q, l1s, l1q):
                    """Stage-interleaved per-sub tail for the final block.
                    The 8 remaining A@V groups get concurrent PSUM banks by
                    borrowing the (now idle) score slots; each stage of the 4
                    sub-chains is spread across Act/DVE/Pool so the chains
                    pipeline."""
                    th = []
                    hold = {}

                    def acc_for(n):
                        # n in 0..7 -> 3 scps tiles (2 banks each) + 2 pa bufs
                        if n < 6:
                            if n % 2 == 0:
                                hold[n // 2] = scps.tile([128, 1024], F32,
                                                         name=f"ta{n}", tag="sc")
                            t = hold[n // 2]
                            return t[:, 0:512] if n % 2 == 0 else t[:, 512:1024]
                        return pa_tile(f"ta{n}")

                    def mk_g(sub, h):
                        n = sub * 2 + (h - 2)
                        def g():
                            av_group(qb, sub, h, acc=acc_for(n))
                        return g

                    ve = nc.vector

                    def mk_stats0(sub):
                        def s0():
                            ti = qb * 4 + sub
                            # combine 4 partials -> l0s[:, sub]
                            ve.tensor_reduce(
                                out=l0s[:, sub:sub + 1],
                                in_=l0p[qb % 2][:, sub * 4:(sub + 1) * 4],
                                axis=mybir.AxisListType.X, op=ALU.add)
                            if sub % 2 == 0:
                                nc.scalar.activation(scr0[:, sub, :], O0n[:, ti, :],
                                                     AF.Square,
                                                     accum_out=l0q[:, sub:sub + 1])
                            else:
                                ve.tensor_tensor_reduce(
                                    out=scr0[:, sub, :], in0=O0n[:, ti, :],
                                    in1=O0n[:, ti, :], scale=1.0, scalar=0.0,
                                    op0=ALU.mult, op1=ALU.add,
                                    accum_out=l0q[:, sub:sub + 1])
                        return s0

                    def mk_fin0(sub):
                        return lambda: ln_fin_sub(
                            O0n, O1n, qb, sub, f"f0{qb}", l0s, l0q,
                            fin_eng=(nc.gpsimd if sub % 2 == 0 else nc.vector),
                            apply_eng=("act" if sub % 2 == 0 else "pool"),
                            store=False)

                    def mk_t1(sub):
                        return lambda: t1(qb * 4 + sub, act_epi=(sub % 2 == 1))

                    def mk_ffn(sub):
                        return lambda: ffn(qb * 4 + sub,
                                           relu_eng=("act" if sub % 2 == 0 else "dve"))

                    def mk_r1s1(sub):
                        def r1():
                            ti = qb * 4 + sub
                            ve.tensor_tensor_reduce(
                                out=O2n[:, ti, :], in0=O1n[:, ti, :],
                                in1=rfn[:, ti, :], scale=1.0, scalar=0.0,
                                op0=ALU.add, op1=ALU.add,
                                accum_out=l1s[:, sub:sub + 1])
                            if sub % 2 == 0:
                                ve.tensor_tensor_reduce(
                                    out=scr1[:, sub, :], in0=O2n[:, ti, :],
                                    in1=O2n[:, ti, :], scale=1.0, scalar=0.0,
                                    op0=ALU.mult, op1=ALU.add,
                                    accum_out=l1q[:, sub:sub + 1])
                            else:
                                nc.scalar.activation(scr1[:, sub, :], O2n[:, ti, :],
                                                     AF.Square,
                                                     accum_out=l1q[:, sub:sub + 1])
                        return r1

                    def mk_fin1(sub):
                        return lambda: ln_fin_sub(
                            O2n, O3n, qb, sub, f"f1{qb}", l1s, l1q,
                            fin_eng=(nc.vector if sub % 2 == 0 else nc.gpsimd),
                            apply_eng=("pool" if sub % 2 == 0 else "act"),
                            store=True)

                    for sub in range(4):
                        th.append((0.3, mk_g(sub, 2)))
                        th.append((0.3, mk_g(sub, 3)))
                    for sub in range(4):
                        th.append((0.3, mk_stats0(sub)))
                    for sub in range(4):
                        th.append((0.3, mk_fin0(sub)))
                    for sub in range(4):
                        th.append((0.3, mk_t1(sub)))
                    for sub in range(4):
                        th.append((0.3, mk_ffn(sub)))
                    for sub in range(4):
                        th.append((0.3, mk_r1s1(sub)))
                    for sub in range(4):
                        th.append((0.3, mk_fin1(sub)))
                    return th

                # ---- PE warmup: dummy matmuls on a memset tile keep the PE
                # continuously busy from ~0.5us so the first real matmuls run
                # at full clock instead of the cold p-state (3.7x slower). ----
                wps = paps.tile([128, 512], F32, name="warm", tag="pa", bufs=2)
                for r in range(34):
                    nc.tensor.matmul(wps[:, 0:128], warm[:], warm[:],
                                     start=True, stop=True)

                # ---- phase A: first projections inline (ramp) ----
                qproj(0, 0, act_epi=True)
                kproj(0, 0, act_epi=True)

                pending = []
                pending.append((0.4, lambda: kproj(0, 1)))
                for kb in (2, 3):
                    pending.append((0.4, lambda kb=kb: kproj(0, kb)))
                for t in range(4):
                    pending.append((0.5, lambda t=t: qnat(t)))
                pending.append((0.4, lambda: qproj(0, 1)))
                for kb in range(KB):
                    pending.append((0.4, lambda kb=kb: kproj(1, kb)))
                for kt in range(KT):
                    pending.append((0.5, lambda kt=kt: vproj(kt)))
                pending.append((0.4, lambda: qproj(1, 0)))
                pending.append((0.4, lambda: qproj(1, 1)))
                for t in range(4, TT):
                    pending.append((0.5, lambda t=t: qnat(t)))

                # --- main loop: dual-engine exp stream with thunk draining ---
                for qb in range(QB):
                    qsl = slice(qb * 512, (qb + 1) * 512)
                    l0p[qb % 2] = smp.tile([128, 16], F32, name=f"l0p{qb}",
                                           tag="l0p", bufs=2)
                    l0s = smp.tile([128, 4], F32, name=f"l0s{qb}", tag="l0s", bufs=2)
                    l0q = smp.tile([128, 4], F32, name=f"l0q{qb}", tag="l0q", bufs=2)
                    l1s = smp.tile([128, 4], F32, name=f"l1s{qb}", tag="l1s", bufs=2)
                    l1q = smp.tile([128, 4], F32, name=f"l1q{qb}", tag="l1q", bufs=2)

                    sc_tiles = {}

                    def emit_sc(i, qsl=qsl, sc_tiles=sc_tiles, qb=qb):
                        hp, kt = divmod(i, KT)
                        sct = scps.tile([128, 1024], F32, name=f"sc{qb}_{i}", tag="sc")
                        for hh in range(2):
                            off = hh * 64
                            nc.tensor.matmul(
                                sct[:, hh * 512:(hh + 1) * 512],
                                kT_bf[off:off + 64, hp, kt * 128:(kt + 1) * 128],
                                qT_bf[off:off + 64, hp, qsl],
                                start=True, stop=True)
                        sc_tiles[i] = sct

                    emit_sc(0)
                    emit_sc(1)
                    for i in range(NT):
                        if i in DVE_TILES:
                            nc.vector.tensor_scalar(
                                uts[:, i, :].bitcast(U8), sc_tiles.pop(i)[:],
                                A_U8, B_U8, ALU.mult, ALU.add)
                        else:
                            nc.scalar.activation(uts[:, i, :], sc_tiles.pop(i)[:],
                                                 AF.Exp, scale=SCALE)
                        if i + 2 < NT:
                            emit_sc(i + 2)
                        if i >= 2 or qb > 0:
                            used = 0.0
                            drained = 0
                            while pending and (drained == 0 or
                                               used + pending[0][0] <= 1.21):
                                c, f = pending.pop(0)
                                f()
                                used += c
                                drained += 1
                        if i == KT:
                            # priority: A@V for heads 0,1 jumps the queue so
                            # the uts tiles free up before the next block's
                            # exps need to overwrite them. Credit 1.2 paces
                            # them one per iteration (the group's DVE
                            # round-trip makes clusters serialize on the pa
                            # bank rotation).
                            pending[0:0] = [
                                (1.2, lambda qb=qb, sub=sub, h=h: av_group(qb, sub, h))
                                for sub in range(4) for h in (0, 1)]
                    last = qb == QB - 1
                    if last and not (use_g0 or use_g1):
                        pending.extend(tail_chain(qb, l0s, l0q, l1s, l1q))
                    else:
                        pending.extend(block_tail(qb, l0s, l0q, l1s, l1q))

                for c, f in pending:
                    f()

    nc.compile()
    return nc


def kernel(Q, K, Wq, bq, Wk, bk, Wv, bv, Wo, bo, g0, b0, g1, b1):
    Q, K = np.asarray(Q), np.asarray(K)
    ws = {n: np.ascontiguousarray(np.asarray(v).T, dtype=np.float32)
          for n, v in (("WqT", Wq), ("WkT", Wk), ("WvT", Wv), ("WoT", Wo))}
    vs = {n: np.ascontiguousarray(np.asarray(v), dtype=np.float32)
          for n, v in (("bq", bq), ("bk", bk), ("bv", bv), ("bo", bo),
                       ("g0", g0), ("b0", b0), ("g1", g1), ("b1", b1))}
    flags = (bool(np.any(vs["bq"])), bool(np.any(vs["bk"])),
             bool(np.any(vs["bv"])), bool(np.any(vs["bo"])),
             bool(np.any(vs["g0"] != 1.0) or np.any(vs["b0"])),
             bool(np.any(vs["g1"] != 1.0) or np.any(vs["b1"])))
    if flags not in _CACHE:
        _CACHE[flags] = _build(flags)
    nc = _CACHE[flags]

    idn = np.eye(128, dtype=np.float32)
    kts = [np.ascontiguousarray(K[b].T, dtype=np.float32) for b in range(B)]
    in_maps = []
    for b in range(B):
        for half in range(2):
            m = {"QT": np.ascontiguousarray(Q[b, half * S:(half + 1) * S].T, dtype=np.float32),
                 "KT": kts[b], "IDN": idn}
            m.update(ws)
            m.update(vs)
            in_maps.append(m)

    res = run_bass_kernel_spmd(nc, in_maps, list(range(8)))
    out = np.empty((B, NQ, D), dtype=np.float32)
    for i in range(8):
        b, half = divmod(i, 2)
        out[b, half * S:(half + 1) * S] = res.results[i]["Out"]
    return out
